# revision 1
# baseline (speedup 1.0000x reference)
"""Trainium2 Bass kernel for nn_Base_Filter (depthwise 7x7 conv + weight-norm +
1x1 projection residual + leaky-decay-relu), sharded over K=1024 channels
across 8 NeuronCores.

Math (folded on host):
  y      = x*(1+w_p) + b_p                       (per-channel affine)
  w_eff  = g * v / ||v||_F                       (weight norm, per channel)
  z      = depthwise_conv7x7_valid(y, w_eff)
  out    = where(z>0, 0.9*z, 0.01*z)

Linearity fold: z = conv(x, w_eff)*(1+w_p) + b_p*sum(w_eff), so with
  w2 = 0.9*(1+w_p)*w_eff,  c2 = 0.9*b_p*sum(w_eff)
we get  out = lrelu(conv(x, w2) + c2, alpha=1/90)  elementwise.

Device kernel (per core, 128 channels on 128 partitions):
  - 49-tap accumulation split three ways (N_PE/N_ACT/N_DVE = 32/12/5):
      TensorE: diagonal-lhsT float32r matmuls (full rate, 1 cycle/row)
               accumulating in PSUM, two 2-row groups per 2-bank tile;
      ScalarE: per-partition-scale multiplies into bf16 temps;
      VectorE: folds the bf16 temps at 2x, runs scalar_tensor_tensor taps,
               and merges the PSUM total.
  - ScalarE applies Lrelu(+bias c2) while evacuating to SBUF.
  - All DMA is contiguous per partition (host pre-transposes x to
    channel-major [1024, 256, 256] and post-transposes the output).
TimelineSim (calibrated cost model): ~897 us/core; engines ~97%/90%/85%
busy (PE/DVE/ACT). HW-verified rel err 2.4e-3 (scale-rel absmax 6.3e-3).
"""

import os
import numpy as np

A = 256
B = 256
R = 32
C = 32
K = 1024
KS = 7
NCORES = 8
P = 128          # channels per core = partitions
AO = A - KS + 1  # 250
BO = B - KS + 1  # 250

H = 24           # output rows per strip
TR = 4           # output rows per PSUM tile (2 banks; matmuls go per 2-row half)
BP = 256         # padded row pitch in PSUM so each 2-row half sits in one bank

# Tap split between TensorE / ScalarE(+VectorE fold) / VectorE (49 total).
N_PE = int(os.environ.get("KRN_N_PE", "32"))
N_ACT = int(os.environ.get("KRN_N_ACT", "12"))
N_DVE = KS * KS - N_PE - N_ACT

_COMPILED = {}
LAST_RESULTS = None  # BassKernelResults of the most recent run (for test.py)


def _build_nc():
    import concourse.bacc as bacc
    import concourse.mybir as mybir
    import concourse.tile as tile

    f32 = mybir.dt.float32
    nc = bacc.Bacc("TRN2", target_bir_lowering=False, debug=False, num_devices=NCORES)

    f32r = mybir.dt.float32r
    x_d = nc.declare_dram_parameter("x", [P, A, B], f32r, isOutput=False)
    dg_d = nc.declare_dram_parameter("dg", [P, max(N_PE, 1), P], f32r, isOutput=False)
    wv_d = nc.declare_dram_parameter(
        "wv", [P, max(N_ACT + N_DVE, 1)], f32, isOutput=False
    )
    c2_d = nc.declare_dram_parameter("c2", [P, 1], f32, isOutput=False)
    out_d = nc.declare_dram_parameter("out", [P, AO, BO], f32, isOutput=True)

    bf16 = mybir.dt.bfloat16
    taps = [(di, dj) for di in range(KS) for dj in range(KS)]
    pe_taps = taps[:N_PE]
    act_taps = taps[N_PE : N_PE + N_ACT]
    dve_taps = taps[N_PE + N_ACT :]

    with tile.TileContext(nc) as tc:
        from contextlib import ExitStack

        with ExitStack() as ctx:
            const = ctx.enter_context(tc.tile_pool(name="const", bufs=1))
            xpool = ctx.enter_context(tc.tile_pool(name="x", bufs=2))
            opool = ctx.enter_context(tc.tile_pool(name="o", bufs=2))
            apool = ctx.enter_context(tc.tile_pool(name="acc", bufs=3))
            bpool = ctx.enter_context(tc.tile_pool(name="accb", bufs=3))
            tpool = ctx.enter_context(tc.tile_pool(name="tmp", bufs=max(N_ACT + 3, 4)))
            ppool = ctx.enter_context(tc.tile_pool(name="ps", bufs=4, space="PSUM"))

            dg_sb = const.tile([P, max(N_PE, 1), P], f32r)
            nc.sync.dma_start(dg_sb[:], dg_d[:])
            wv_sb = const.tile([P, max(N_ACT + N_DVE, 1)], f32)
            nc.sync.dma_start(wv_sb[:], wv_d[:])
            c2_sb = const.tile([P, 1], f32)
            nc.sync.dma_start(c2_sb[:], c2_d[:])

            row0 = 0
            while row0 < AO:
                rows = min(H, AO - row0)
                in_rows = rows + KS - 1
                xs = xpool.tile([P, in_rows, B], f32r, tag="xs")
                nc.sync.dma_start(xs[:], x_d[:, row0 : row0 + in_rows, :])
                outs = opool.tile([P, rows, BO], f32, tag="outs")

                o0 = 0
                while o0 < rows:
                    tr = min(TR, rows - o0)
                    ps = ppool.tile([P, TR, BP], f32, tag="ps")
                    for h in range(0, tr, 2):
                        hr = min(2, tr - h)
                        out_ap = ps[:, h : h + hr, 0:BO]
                        for i, (di, dj) in enumerate(pe_taps):
                            rhs = xs[:, o0 + h + di : o0 + h + di + hr, dj : dj + BO]
                            # float32r: full-rate (1 cycle/row) fp32 matmul
                            nc.tensor.matmul(
                                out_ap,
                                dg_sb[:, i, :],
                                rhs,
                                start=(i == 0),
                                stop=(i == len(pe_taps) - 1),
                            )
                    ps_ap = ps[:, 0:tr, 0:BO]

                    # ScalarE-assist taps: tmp_m = x_win * w  (bf16), folded
                    # pairwise on VectorE at bf16 2x rate into accb.
                    accb_ap = None
                    if N_ACT > 0:
                        accb = bpool.tile([P, TR, BP], bf16, tag="accb")
                        accb_ap = accb[:, 0:tr, 0:BO]
                        tmps = []
                        for m, (di, dj) in enumerate(act_taps):
                            rhs = xs[
                                :, o0 + di : o0 + di + tr, dj : dj + BO
                            ].bitcast(f32)
                            tmp = tpool.tile([P, TR, BP], bf16, tag="tmp")
                            nc.scalar.mul(
                                tmp[:, 0:tr, 0:BO], rhs, wv_sb[:, m : m + 1]
                            )
                            tmps.append(tmp[:, 0:tr, 0:BO])
                        nc.vector.tensor_tensor(
                            accb_ap, tmps[0], tmps[1], mybir.AluOpType.add
                        )
                        for m in range(2, N_ACT):
                            nc.vector.tensor_tensor(
                                accb_ap, accb_ap, tmps[m], mybir.AluOpType.add
                            )

                    # VectorE stt taps first (independent of PSUM), then
                    # fold accb and the PSUM total at the end.
                    acc = apool.tile([P, TR, BO], f32, tag="acc")
                    acc_ap = acc[:, 0:tr, :]
                    for j, (di, dj) in enumerate(dve_taps):
                        rhs = xs[
                            :, o0 + di : o0 + di + tr, dj : dj + BO
                        ].bitcast(f32)
                        if j == 0:
                            nc.vector.tensor_scalar(
                                acc_ap,
                                rhs,
                                wv_sb[:, N_ACT : N_ACT + 1],
                                None,
                                mybir.AluOpType.mult,
                            )
                        else:
                            nc.vector.scalar_tensor_tensor(
                                acc_ap,
                                rhs,
                                wv_sb[:, N_ACT + j : N_ACT + j + 1],
                                acc_ap,
                                mybir.AluOpType.mult,
                                mybir.AluOpType.add,
                            )
                    if accb_ap is not None:
                        nc.vector.tensor_tensor(
                            acc_ap, acc_ap, accb_ap, mybir.AluOpType.add
                        )
                    nc.vector.tensor_tensor(
                        acc_ap, acc_ap, ps_ap, mybir.AluOpType.add
                    )
                    src = acc_ap
                    # out = lrelu(src + c2), alpha = 0.01/0.9
                    nc.scalar.activation(
                        outs[:, o0 : o0 + tr, :],
                        src,
                        mybir.ActivationFunctionType.Lrelu,
                        bias=c2_sb[:, 0:1],
                        scale=1.0,
                        alpha=0.01 / 0.9,
                    )
                    o0 += tr

                nc.sync.dma_start(out_d[:, row0 : row0 + rows, :], outs[:])
                row0 += rows

    nc.compile()
    return nc


def _prep_weights(w_p, b_p, v, g):
    v = v.astype(np.float32)
    v_norm = np.sqrt((v * v).sum(axis=(1, 2), keepdims=True))
    w_eff = g[:, None, None].astype(np.float32) * v / v_norm          # [K,7,7]
    w2 = 0.9 * (1.0 + w_p)[:, None, None].astype(np.float32) * w_eff  # [K,7,7]
    c2 = (0.9 * b_p.astype(np.float32) * w_eff.sum(axis=(1, 2)))      # [K]
    return w2.astype(np.float32), c2.astype(np.float32)


def kernel(x, w_p, b_p, v, g):
    global LAST_RESULTS
    from concourse.bass_utils import run_bass_kernel_spmd

    x = np.asarray(x, dtype=np.float32)
    w2, c2 = _prep_weights(
        np.asarray(w_p, np.float32),
        np.asarray(b_p, np.float32),
        np.asarray(v, np.float32),
        np.asarray(g, np.float32),
    )

    # channel-major x: [K, A, B], k = r*C + c (matches reference's kernel_index)
    x_t = np.ascontiguousarray(x.transpose(2, 3, 0, 1).reshape(K, A, B))

    taps = [(di, dj) for di in range(KS) for dj in range(KS)]
    in_maps = []
    ar = np.arange(P)
    for core in range(NCORES):
        sl = slice(core * P, (core + 1) * P)
        w2c = w2[sl]  # [P,7,7]
        dg = np.zeros((max(N_PE, 1), P, P), dtype=np.float32)
        for i, (di, dj) in enumerate(taps[:N_PE]):
            dg[i, ar, ar] = w2c[:, di, dj]
        # SBUF layout [P, N_PE, P]: dg_sb[p, t, m] = dg[t, p, m]
        dg_sb = np.ascontiguousarray(dg.transpose(1, 0, 2))
        wv = np.zeros((P, max(N_ACT + N_DVE, 1)), dtype=np.float32)
        for j, (di, dj) in enumerate(taps[N_PE:]):
            wv[:, j] = w2c[:, di, dj]
        in_maps.append(
            {
                "x": np.ascontiguousarray(x_t[sl]),
                "dg": dg_sb,
                "wv": wv,
                "c2": np.ascontiguousarray(c2[sl][:, None]),
            }
        )

    assert N_DVE >= 1 and N_PE >= 1
    key = ("v1", N_PE, N_ACT)
    if key not in _COMPILED:
        _COMPILED[key] = _build_nc()
    nc = _COMPILED[key]

    trace = os.environ.get("KRN_TRACE", "0") == "1"
    res = run_bass_kernel_spmd(nc, in_maps, list(range(NCORES)), trace=trace)
    LAST_RESULTS = res

    out_full = np.empty((K, AO, BO), dtype=np.float32)
    for core in range(NCORES):
        out_full[core * P : (core + 1) * P] = res.results[core]["out"]

    # [K, AO, BO] -> [AO, BO, R, C]
    return np.ascontiguousarray(
        out_full.reshape(R, C, AO, BO).transpose(2, 3, 0, 1)
    )


if __name__ == "__main__":
    rng = np.random.default_rng(0)
    xs = rng.standard_normal((A, B, R, C), dtype=np.float32)
    out = kernel(
        xs,
        rng.standard_normal(K).astype(np.float32) * 0.1,
        rng.standard_normal(K).astype(np.float32) * 0.1,
        rng.standard_normal((K, KS, KS)).astype(np.float32),
        rng.standard_normal(K).astype(np.float32),
    )
    print(out.shape, out.dtype)

